# revision 17
# baseline (speedup 1.0000x reference)
"""Trainium2 Bass kernel for the per-game CriticNetwork (MoE-routed MLP).

Network (per sample b, with game g = idx[b]):
    h1  = relu(W1[g] @ state[b] + b1[g])          # [600]
    h2  = W2s @ h1 + b2s + W2a[g] @ action[b]     # [500]
    q   = W3[g] . relu(h2) + b3[g]                # scalar

Strategy: all MoE routing happens on the HOST. idx is (stably) sorted into
per-game contiguous segments, each segment is padded up to 512-sample tiles,
and the tile list is padded to a fixed 72 tiles (9 per core x 8 cores).
Every tile is single-game, so the device kernel is a fully static dense
pipeline; the host pre-gathers per-tile weight views (pre-transposed for the
PE's lhsT layout) so the device does zero routing and zero transposes.

Precision/speed layout (per 512-sample tile):
  L1 (state->h1 pre-act) runs in bf16: 5 matmuls [K=128, M=128, N=512].
      W1 is host-scaled by SH=32 (exact power of 2 in bf16) so the relu
      output 32*h1 lands in fp8e4m3's normal range.
  r1: relu+bias (fp32 PSUM -> fp8 SBUF) on ACT/DVE; c0..c3 fill the h1
      slab [128, 4, 512], c4 lands in the w8 tile's k-tile-4 slot. (Only
      ACT/DVE can read PSUM, so they are the throughput wall: ~2.9us/tile
      of relu+convert work vs ~2.35us of PE work.)
  L2 (h1->h2) runs in fp8 DoubleRow perf mode (0.5 PE cycles/row, 2 k-tiles
      of 128 per instruction): per m-chunk 3 DR matmuls with
      lhsT = fp8(64*W2s^T) k-tile pairs (0,1), (2,3), (action-block, kt4).
      The action term is folded in with first-order error compensation:
      action lhsT rows = [Whi;Whi;Wlo;64*b2s] vs rhs rows [ahi;alo;ahi;32.0]
      where Whi/Wlo = fp8 hi/lo split of 64*W2a[g]^T and ahi/alo of
      32*action; the last row folds b2s into the chain so the hf relus need
      no bias. PSUM accumulates 2048*h2.
  rhf: plain relu -> hf in bf16 (scaled by 2048; exact), one op per
      m-chunk, column-split across ACT/DVE per CFG to balance the engines
      (A ~2.93us/tile = V ~2.93us/tile = the steady-state period).
  L3: 2048*q = W3^T(bf16) . hf: 16 matmuls with lhsT = hf chunk [K=128,
      M=128 samples], rhs = one W3 column [K=128, N=1] -> out free size 1,
      accumulated into one persistent PSUM tile [128, 4*NT], drained at end.
  Host divides by 2048 and adds b3.

DMA per tile (merged to keep the serialized HWDGE generator off the
critical path): cb = [stateT | 32*W1^T | W3 cols] bf16 in one transfer,
w8 = [action-rows | action-lhsT | W2s-kt4-lhsT] fp8 in one transfer.
Shared fp8 W2s k-tiles 0..3 and the b1 biases load once.

The emission order is a software-pipelined token schedule (CFG["template"]):
tile t's L2 pairs p1/p2, hf relus and L3 run during tile t+1's L1 phase.
Measured rel err vs the fp32 reference: ~1.09e-2 (gate is 2e-2).
"""

import numpy as np

import concourse.bass as bass
import concourse.mybir as mybir
import concourse.tile as tile
from concourse import bacc
from concourse.bass import ts
from concourse.bass_utils import run_bass_kernel_spmd

F32 = mybir.dt.float32
BF16 = mybir.dt.bfloat16
FP8 = mybir.dt.float8e4
RELU = mybir.ActivationFunctionType.Relu
DR = mybir.MatmulPerfMode.DoubleRow

_NP_BF16 = mybir.dt.np(BF16)
_NP_FP8 = mybir.dt.np(FP8)

G = 8          # games
D = 128        # state dim
A = 16         # action dim
H1 = 600       # hidden 1 (padded to 640 = 5 * 128)
H2 = 500       # hidden 2 (padded to 512 = 4 * 128)
B = 32768      # batch
H1P, H2P = 640, 512
K1 = H1P // 128   # 5 h1 chunks
M2 = H2P // 128   # 4 h2 chunks
NP2 = 3           # DR k-tile pairs per m-chunk
T = 512        # samples per tile (one PSUM bank of fp32)
NCORES = 8
NT = 9         # tiles per core; 72 total >= 64 + 7 worst-case segment padding
BPC = NT * T   # 4608 lanes per core

SH = 32.0      # h1 / action scale (power of 2)
SW = 64.0      # W2s / W2a / b2s scale (power of 2)
SP2 = SH * SW  # h2 PSUM scale

# cb blob [128, CB] (bf16): cols 0:512 stateT tile, 512:1152 w1t, 1152:1156 w3
STOFF = 0
W1OFF = T
W3OFF = T + H1P
CB = T + H1P + M2

# Token schedule per tile t (software-pipelined; "p" tokens refer to t-1):
#   ("l1", c)        L1 matmul chunk c
#   ("r1", c)        relu for L1 chunk c (c4 = DMA bounce + Pool relu)
#   ("l2", p, ms)    L2 DR pair-p matmuls for m in ms (this tile)
#   ("l2p", p, ms)   same, previous tile
#   ("rhfp", j)      hf relu pair j (m 2j, 2j+1), previous tile
#   ("l3p", s)       L3 slice chain s, previous tile
# Engines: 'A' = ACT, 'V' = DVE.
CFG = {
    "ps1_bufs": 3,
    "ps2_bufs": 4,
    "r1_eng": {0: 'A', 1: 'V', 2: 'A', 3: 'V', 4: 'A'},
    # rhf m2 split: columns 0:344 on ACT, 344:512 on DVE
    "rhf_eng": {0: 'A', 1: 'V', 2: ('A', 344, 'V'), 3: 'V'},
    "flush_rhf_eng": {0: 'A', 1: 'V', 2: 'A', 3: 'V'},
    "qb_eng": 'A',
    "template": [
        ("l1", 0), ("r1", 0),
        ("l2p", 1, (0, 1, 2, 3)),
        ("l1", 1), ("r1", 1),
        ("l2p", 2, (0, 1, 2, 3)),
        ("l1", 2), ("r1", 2),
        ("rhfp", 1), ("rhfp", 0),
        ("l1", 3), ("r1", 3),
        ("rhfp", 3), ("rhfp", 2),
        ("l1", 4), ("r1", 4),
        ("l2", 0, (0, 1, 2, 3)),
        ("l3p", 0), ("l3p", 1), ("l3p", 2), ("l3p", 3),
    ],
    "q_eng": 'V',        # final psq->sbuf copy engine
    "t0_dma": ("cb0a", "cb0b", "w8P", "bc", "w2s8"),
    "pe_warm": True,
    "early_drain": True,
    "t_dma": ("cb", "w8"),
}
# The last tile has no successor phase: drain it inside its own phase.
CFG["template_last"] = CFG["template"] + [
    ("qdA",),
    ("l2", 1, (0, 1, 2, 3)),
    ("l2", 2, (0, 1, 2, 3)),
    ("rhff", 0), ("rhff", 1), ("rhff", 2), ("rhff", 3),
    ("l3", 0), ("l3", 1), ("l3", 2), ("l3", 3),
]

_NC = None


def build_nc():
    nc = bacc.Bacc("TRN2", target_bir_lowering=False, debug=False,
                   num_devices=NCORES)

    comb16 = nc.declare_dram_parameter("comb16", [NT, 128, CB], BF16,
                                       isOutput=False)
    # per-tile fp8 blob [128, 3, 512]: sub0 = action rows (rhs k-tile),
    # sub1 = action lhsT block, sub2 = W2s k-tile-4 lhsT
    comb8 = nc.declare_dram_parameter("comb8", [NT, 128, NP2, T], FP8,
                                      isOutput=False)
    # shared L2 pairs 0,1 lhsT: [m, p(2), i(2), col(128)]
    w2s8 = nc.declare_dram_parameter("w2s8", [128, M2, 2, 2, 128], FP8,
                                     isOutput=False)
    # b1 biases (*SH): cols t*K1+c
    bconst = nc.declare_dram_parameter("bconst", [128, NT * K1], F32,
                                       isOutput=False)
    # q[j, 4t+s] = 2048*q of lane 512t + 128s + j
    q = nc.declare_dram_parameter("q", [128, M2 * NT], F32, isOutput=True)

    eng = {'A': lambda: nc.scalar, 'V': lambda: nc.vector,
           'P': lambda: nc.gpsimd}

    with tile.TileContext(nc) as tc:
        with (
            tc.tile_pool(name="const", bufs=1) as const,
            tc.tile_pool(name="wts", bufs=4) as wts,
            tc.tile_pool(name="hpool", bufs=3) as hpool,
            tc.tile_pool(name="outp", bufs=1) as outp,
            tc.tile_pool(name="ps1", bufs=CFG["ps1_bufs"], space="PSUM") as ps1p,
            tc.tile_pool(name="ps2", bufs=CFG["ps2_bufs"], space="PSUM") as ps2p,
            tc.tile_pool(name="psq", bufs=1, space="PSUM") as psqp,
        ):
            w2s8_sb = const.tile([128, M2, 2, 2, 128], FP8)
            bc_sb = const.tile([128, NT * K1], F32)
            psq = psqp.tile([128, M2 * NT], F32)
            # Warm-up: trigger the ACT-table load (~1.3us) during the
            # initial DMA dead time instead of before the first real relu.
            dumt = const.tile([1, 4], F32)
            nc.gpsimd.memset(dumt[:], 0.0)
            nc.scalar.activation(dumt[:, 0:1], dumt[:, 1:2], RELU, bias=0.0)
            if CFG.get("pe_warm"):
                nc.tensor.matmul(psq[0:1, 0:1], dumt[0:1, 2:3],
                                 dumt[0:1, 3:4], start=True, stop=True)

            def relu_op(e, out, in_, bias):
                if e == 'A':
                    nc.scalar.activation(out, in_, RELU, bias=bias)
                elif isinstance(bias, float):
                    eng[e]().tensor_scalar_max(out, in_, bias)
                else:
                    eng[e]().tensor_scalar(out, in_, bias, 0.0,
                                           mybir.AluOpType.add,
                                           mybir.AluOpType.max)

            class Tile:
                def __init__(self, t):
                    self.t = t
                    self.cb = wts.tile([128, CB], BF16, tag="cb")
                    # w8: subs 0:3 DMA'd (action rows, action lhsT, W2s kt4);
                    # sub 3 = 32*h1 k-tile 4, written by the Pool relu.
                    self.w8 = wts.tile([128, NP2 + 1, T], FP8, tag="w8")
                    self.h1 = hpool.tile([128, 4, T], FP8, tag="h1")
                    dmas = {
                        "cb": lambda: nc.sync.dma_start(self.cb[:], comb16[t]),
                        "cb0a": lambda: nc.sync.dma_start(
                            self.cb[:, 0:W1OFF + 128],
                            comb16[t][:, 0:W1OFF + 128]),
                        "cb0b": lambda: nc.sync.dma_start(
                            self.cb[:, W1OFF + 128:],
                            comb16[t][:, W1OFF + 128:]),
                        "w8": lambda: nc.sync.dma_start(
                            self.w8[:, 0:NP2, :], comb8[t]),
                        "w8P": lambda: nc.gpsimd.dma_start(
                            self.w8[:, 0:NP2, :], comb8[t]),
                        "w2s8": lambda: nc.sync.dma_start(
                            w2s8_sb[:], w2s8.ap()),
                        "bc": lambda: nc.sync.dma_start(bc_sb[:], bconst.ap()),
                    }
                    order = CFG["t0_dma"] if t == 0 else CFG["t_dma"]
                    for d in order:
                        dmas[d]()
                    self.hf = hpool.tile([128, M2, T], BF16, tag="hf")
                    self.ps1 = {}
                    self.ps2 = {}

                def l1(self, c):
                    ps1c = ps1p.tile([128, T], F32, tag="ps1")
                    p = self.ps1[c] = ps1c
                    nc.tensor.matmul(p[:], self.cb[:, W1OFF + 128 * c:
                                                   W1OFF + 128 * (c + 1)],
                                     self.cb[:, STOFF:STOFF + T],
                                     start=True, stop=True)

                def r1(self, c, e):
                    p = self.ps1.pop(c)
                    b = bc_sb[:, self.t * K1 + c:self.t * K1 + c + 1]
                    out = (self.w8[:, NP2, :] if c == K1 - 1
                           else self.h1[:, c, :])
                    relu_op(e, out, p[:], b)

                def l2(self, p, ms):
                    for m in ms:
                        if p == 0:
                            ps2m = ps2p.tile([128, T], F32, tag="ps2")
                            self.ps2[m] = ps2m
                        if p == NP2 - 1:
                            lhsT = self.w8[:, 1:NP2, 128 * m:128 * (m + 1)]
                            rhs = self.w8[:, 0:NP2 + 1:NP2, :]
                        else:
                            lhsT = w2s8_sb[:, m, p, :, :]
                            rhs = self.h1[:, 2 * p:2 * p + 2, :]
                        nc.tensor.matmul(self.ps2[m][:], lhsT, rhs,
                                         start=(p == 0), stop=(p == NP2 - 1),
                                         perf_mode=DR)

                def rhf(self, m, e):
                    p = self.ps2.pop(m)
                    hfm = self.hf[:, m, :]
                    if isinstance(e, tuple):
                        e0, h, e1 = e
                        relu_op(e0, hfm[:, 0:h], p[:, 0:h], 0.0)
                        relu_op(e1, hfm[:, h:T], p[:, h:T], 0.0)
                    else:
                        relu_op(e, hfm, p[:], 0.0)

                def l3(self, s):
                    col = M2 * self.t + s
                    for m in range(M2):
                        nc.tensor.matmul(psq[:, col:col + 1],
                                         self.hf[:, m, ts(s, 128)],
                                         self.cb[:, W3OFF + m:W3OFF + m + 1],
                                         start=(m == 0), stop=(m == M2 - 1))

            q_sb = outp.tile([128, M2 * NT], F32, tag="q")
            ned = M2 * (NT - 1)

            def drain_a():
                eng[CFG["q_eng"]]().tensor_copy(q_sb[:, 0:ned],
                                                psq[:, 0:ned])
                nc.sync.dma_start(q.ap()[:, 0:ned], q_sb[:, 0:ned])

            def run_token(tok, cur, prev, flush=False):
                kind = tok[0]
                if kind == "l1":
                    cur.l1(tok[1])
                elif kind == "r1":
                    cur.r1(tok[1], CFG["r1_eng"].get(tok[1]))
                elif kind == "l2":
                    cur.l2(tok[1], tok[2])
                elif kind == "l2p":
                    if prev is not None:
                        prev.l2(tok[1], tok[2])
                elif kind == "rhf":
                    cur.rhf(tok[1], CFG["rhf_eng"][tok[1]])
                elif kind == "rhff":
                    cur.rhf(tok[1], CFG["flush_rhf_eng"][tok[1]])
                elif kind == "l3":
                    cur.l3(tok[1])
                elif kind == "rhfp":
                    if prev is not None:
                        emap = CFG["flush_rhf_eng"] if flush else CFG["rhf_eng"]
                        prev.rhf(tok[1], emap[tok[1]])
                elif kind == "l3p":
                    if prev is not None:
                        prev.l3(tok[1])
                elif kind == "qdA":
                    drain_a()
                else:
                    raise ValueError(tok)

            prev = None
            for t in range(NT):
                cur = Tile(t)
                tmpl = CFG["template"]
                if t == NT - 1 and CFG.get("template_last"):
                    tmpl = CFG["template_last"]
                for tok in tmpl:
                    run_token(tok, cur, prev)
                prev = cur
            e = CFG["qb_eng"]
            if e == 'A':
                nc.scalar.activation(q_sb[:, ned:], psq[:, ned:],
                                     mybir.ActivationFunctionType.Copy)
            else:
                eng[e]().tensor_copy(q_sb[:, ned:], psq[:, ned:])
            nc.sync.dma_start(q.ap()[:, ned:], q_sb[:, ned:])

    nc.compile()
    return nc


def _get_nc():
    global _NC
    if _NC is None:
        _NC = build_nc()
    return _NC


def _plan_tiles(idx):
    """Stable-sort samples by game, pad each game segment to 512-sample
    tiles, pad the tile list to the fixed 72. Returns (sel, valid, gids):
    sel[t, l] = original sample index feeding lane l of tile t."""
    perm = np.argsort(idx, kind="stable")
    counts = np.bincount(idx, minlength=G)
    ntot = NCORES * NT
    sel = np.zeros((ntot, T), np.int64)
    valid = np.zeros((ntot, T), bool)
    gids = np.zeros(ntot, np.int64)
    pos, t = 0, 0
    for g in range(G):
        cg = int(counts[g])
        for k in range((cg + T - 1) // T):
            n = min(T, cg - k * T)
            lanes = perm[pos:pos + n]
            sel[t, :n] = lanes
            valid[t, :n] = True
            if n < T:
                sel[t, n:] = lanes[0]
            gids[t] = g
            pos += n
            t += 1
    assert t <= ntot, f"tile plan overflow: {t} > {ntot}"
    return sel, valid, gids


def _f8(x):
    return np.asarray(x, np.float32).astype(_NP_FP8)


def build_in_maps(inputs):
    state = np.ascontiguousarray(np.asarray(inputs["state"], np.float32))
    action = np.ascontiguousarray(np.asarray(inputs["action"], np.float32))
    idx = np.asarray(inputs["idx"]).astype(np.int64)
    W1 = np.asarray(inputs["W1"], np.float32)
    b1 = np.asarray(inputs["b1"], np.float32)
    W2s = np.asarray(inputs["W2s"], np.float32)
    b2s = np.asarray(inputs["b2s"], np.float32)
    W2a = np.asarray(inputs["W2a"], np.float32)
    W3 = np.asarray(inputs["W3"], np.float32)
    assert state.shape == (B, D) and action.shape == (B, A)

    sel, valid, gids = _plan_tiles(idx)

    # L1 weights: 32*W1^T in bf16, padded 600->640.
    W1T_all = np.zeros((G, D, H1P), np.float32)
    W1T_all[:, :, :H1] = W1.transpose(0, 2, 1) * SH
    b1P = np.zeros((G, H1P), np.float32)
    b1P[:, :H1] = b1 * SH
    b1c_all = np.ascontiguousarray(b1P.reshape(G, K1, 128).transpose(0, 2, 1))

    # L2 shared weights: fp8(64*W2s^T), k-tiles arranged per (m, pair, i).
    W2sTP = np.zeros((H1P, H2P), np.float32)
    W2sTP[:H1, :H2] = W2s.T * SW
    W2sq = _f8(W2sTP)  # [640, 512]
    w2skt = W2sq.reshape(K1, 128, M2, 128)
    w2s8_host = np.zeros((128, M2, 2, 2, 128), _NP_FP8)
    for p in range(2):
        for i in range(2):
            w2s8_host[:, :, p, i, :] = w2skt[2 * p + i]

    # Per-game action lhsT block rows [Whi(16); Whi(16); Wlo(16); 64*b2s].
    W2aT_all = np.zeros((G, A, H2P), np.float32)
    W2aT_all[:, :, :H2] = W2a.transpose(0, 2, 1) * SW
    Wahi = _f8(W2aT_all)
    Walo = _f8(W2aT_all - Wahi.astype(np.float32))
    b2sP = np.zeros(H2P, np.float32)
    b2sP[:H2] = b2s * SW
    b2sq = _f8(b2sP)
    act_lhsT = np.zeros((G, 128, T), _NP_FP8)
    act_lhsT[:, 0:A] = Wahi
    act_lhsT[:, A:2 * A] = Wahi
    act_lhsT[:, 2 * A:3 * A] = Walo
    act_lhsT[:, 3 * A] = b2sq[None, :]

    W3P = np.zeros((G, H2P), np.float32)
    W3P[:, :H2] = W3
    W3T_all = np.ascontiguousarray(W3P.reshape(G, M2, 128).transpose(0, 2, 1))

    # cb per-game static part: [128, w1t | w3t] bf16 (state filled per tile)
    wb16_all = np.zeros((G, 128, H1P + M2), np.float32)
    wb16_all[:, :, 0:H1P] = W1T_all
    wb16_all[:, :, H1P:] = W3T_all
    wb16_all = wb16_all.astype(_NP_BF16)

    # Action rows (per sample): [ahi; alo; ahi; 32.0; 0...] of 32*action^T.
    aS = action * SH
    ahi_all = _f8(aS)                                    # [B, 16]
    alo_all = _f8(aS - ahi_all.astype(np.float32))       # [B, 16]

    in_maps = []
    for c in range(NCORES):
        tsl = slice(c * NT, (c + 1) * NT)
        lanes = sel[tsl].reshape(-1)
        gt = gids[tsl]
        bconst = np.ascontiguousarray(
            b1c_all[gt].transpose(1, 0, 2).reshape(128, NT * K1))
        comb16_host = np.zeros((NT, 128, CB), _NP_BF16)
        comb16_host[:, :, STOFF:STOFF + T] = (
            state[lanes].T.reshape(128, NT, T).transpose(1, 0, 2)
            .astype(_NP_BF16))
        comb16_host[:, :, W1OFF:] = wb16_all[gt]
        comb8_host = np.zeros((NT, 128, NP2, T), _NP_FP8)
        ahi_c = ahi_all[lanes].reshape(NT, T, A)
        alo_c = alo_all[lanes].reshape(NT, T, A)
        for t in range(NT):
            comb8_host[t, 0:A, 0] = ahi_c[t].T
            comb8_host[t, A:2 * A, 0] = alo_c[t].T
            comb8_host[t, 2 * A:3 * A, 0] = ahi_c[t].T
            comb8_host[t, 3 * A, 0] = np.float32(SH)
        comb8_host[:, :, 1] = act_lhsT[gt]
        comb8_host[:, :, 2] = w2skt[4].reshape(128, T)[None]
        in_maps.append({
            "comb16": comb16_host,
            "comb8": comb8_host,
            "w2s8": w2s8_host,
            "bconst": bconst,
        })
    return in_maps, sel, valid


def kernel(**inputs):
    idx = np.asarray(inputs["idx"]).astype(np.int64)
    b3 = np.asarray(inputs["b3"], np.float32)
    in_maps, sel, valid = build_in_maps(inputs)

    res = run_bass_kernel_spmd(_get_nc(), in_maps, list(range(NCORES))).results
    # q[j, 4t+s] = lane 512t + 128s + j  ->  [t, s, j] order
    qv = np.concatenate([
        np.asarray(res[c]["q"]).reshape(128, NT, M2).transpose(1, 2, 0)
        .reshape(-1)
        for c in range(NCORES)])

    out = np.zeros(B, np.float32)
    flat_sel = sel.reshape(-1)
    flat_valid = valid.reshape(-1)
    out[flat_sel[flat_valid]] = qv[flat_valid] / SP2
    out += b3[idx]
    return out.astype(np.float32)


# revision 19
# speedup vs baseline: 1.0033x; 1.0033x over previous
"""Trainium2 Bass kernel for the per-game CriticNetwork (MoE-routed MLP).

Network (per sample b, with game g = idx[b]):
    h1  = relu(W1[g] @ state[b] + b1[g])          # [600]
    h2  = W2s @ h1 + b2s + W2a[g] @ action[b]     # [500]
    q   = W3[g] . relu(h2) + b3[g]                # scalar

Strategy: all MoE routing happens on the HOST. idx is (stably) sorted into
per-game contiguous segments, each segment is padded up to 512-sample tiles,
and the tile list is padded to a fixed 72 tiles (9 per core x 8 cores).
Every tile is single-game, so the device kernel is a fully static dense
pipeline; the host pre-gathers per-tile weight views (pre-transposed for the
PE's lhsT layout) so the device does zero routing and zero transposes.

Precision/speed layout (per 512-sample tile):
  L1 (state->h1 pre-act) runs in bf16: 5 matmuls [K=128, M=128, N=512].
      W1 is host-scaled by SH=32 (exact power of 2 in bf16) so the relu
      output 32*h1 lands in fp8e4m3's normal range.
  r1: relu+bias (fp32 PSUM -> fp8 SBUF) on ACT/DVE; c0..c3 fill the h1
      slab [128, 4, 512], c4 lands in the w8 tile's k-tile-4 slot. (Only
      ACT/DVE can read PSUM, so they are the throughput wall: ~2.9us/tile
      of relu+convert work vs ~2.35us of PE work.)
  L2 (h1->h2) runs in fp8 DoubleRow perf mode (0.5 PE cycles/row, 2 k-tiles
      of 128 per instruction): per m-chunk 3 DR matmuls with
      lhsT = fp8(64*W2s^T) k-tile pairs (0,1), (2,3), (action-block, kt4).
      The action term is folded in with first-order error compensation:
      action lhsT rows = [Whi;Whi;Wlo;64*b2s] vs rhs rows [ahi;alo;ahi;32.0]
      where Whi/Wlo = fp8 hi/lo split of 64*W2a[g]^T and ahi/alo of
      32*action; the last row folds b2s into the chain so the hf relus need
      no bias. PSUM accumulates 2048*h2.
  rhf: plain relu -> hf in bf16 (scaled by 2048; exact), one op per
      m-chunk, column-split across ACT/DVE per CFG to balance the engines
      (A ~2.93us/tile = V ~2.93us/tile = the steady-state period).
  L3: 2048*q = W3^T(bf16) . hf: 16 matmuls with lhsT = hf chunk [K=128,
      M=128 samples], rhs = one W3 column [K=128, N=1] -> out free size 1,
      accumulated into one persistent PSUM tile [128, 4*NT], drained at end.
  Host divides by 2048 and adds b3.

DMA per tile (merged to keep the serialized HWDGE generator off the
critical path): cb = [stateT | 32*W1^T | W3 cols] bf16 in one transfer,
w8 = [action-rows | action-lhsT | W2s-kt4-lhsT] fp8 in one transfer.
Shared fp8 W2s k-tiles 0..3 and the b1 biases load once.

The emission order is a software-pipelined token schedule (CFG["template"]):
tile t's L2 pairs p1/p2, hf relus and L3 run during tile t+1's L1 phase.
Measured rel err vs the fp32 reference: ~1.09e-2 (gate is 2e-2).
"""

import numpy as np

import concourse.bass as bass
import concourse.mybir as mybir
import concourse.tile as tile
from concourse import bacc
from concourse.bass import ts
from concourse.bass_utils import run_bass_kernel_spmd

F32 = mybir.dt.float32
BF16 = mybir.dt.bfloat16
FP8 = mybir.dt.float8e4
RELU = mybir.ActivationFunctionType.Relu
DR = mybir.MatmulPerfMode.DoubleRow

_NP_BF16 = mybir.dt.np(BF16)
_NP_FP8 = mybir.dt.np(FP8)

G = 8          # games
D = 128        # state dim
A = 16         # action dim
H1 = 600       # hidden 1 (padded to 640 = 5 * 128)
H2 = 500       # hidden 2 (padded to 512 = 4 * 128)
B = 32768      # batch
H1P, H2P = 640, 512
K1 = H1P // 128   # 5 h1 chunks
M2 = H2P // 128   # 4 h2 chunks
NP2 = 3           # DR k-tile pairs per m-chunk
T = 512        # samples per tile (one PSUM bank of fp32)
NCORES = 8
NT = 9         # tiles per core; 72 total >= 64 + 7 worst-case segment padding
BPC = NT * T   # 4608 lanes per core

SH = 32.0      # h1 / action scale (power of 2)
SW = 64.0      # W2s / W2a / b2s scale (power of 2)
SP2 = SH * SW  # h2 PSUM scale

# cb blob [128, CB] (bf16): cols 0:512 stateT tile, 512:1152 w1t, 1152:1156 w3
STOFF = 0
W1OFF = T
W3OFF = T + H1P
CB = T + H1P + M2

# Token schedule per tile t (software-pipelined; "p" tokens refer to t-1):
#   ("l1", c)        L1 matmul chunk c
#   ("r1", c)        relu for L1 chunk c (c4 = DMA bounce + Pool relu)
#   ("l2", p, ms)    L2 DR pair-p matmuls for m in ms (this tile)
#   ("l2p", p, ms)   same, previous tile
#   ("rhfp", j)      hf relu pair j (m 2j, 2j+1), previous tile
#   ("l3p", s)       L3 slice chain s, previous tile
# Engines: 'A' = ACT, 'V' = DVE.
CFG = {
    "ps1_bufs": 3,
    "ps2_bufs": 4,
    "r1_eng": {0: 'A', 1: 'V', 2: 'A', 3: 'V', 4: 'A'},
    # rhf m2 split: columns 0:344 on ACT, 344:512 on DVE
    "rhf_eng": {0: 'A', 1: 'V', 2: ('A', 344, 'V'), 3: 'V'},
    "flush_rhf_eng": {0: 'A', 1: 'V', 2: 'A', 3: 'V'},  # rhff engine map
    "qb_eng": 'V',
    "template": [
        ("l1", 0), ("r1", 0),
        ("l2p", 1, (0, 1, 2, 3)),
        ("l1", 1), ("r1", 1),
        ("l2p", 2, (0, 1, 2, 3)),
        ("l1", 2), ("r1", 2),
        ("rhfp", 1), ("rhfp", 0),
        ("l1", 3), ("r1", 3),
        ("rhfp", 3), ("rhfp", 2),
        ("l1", 4), ("r1", 4),
        ("l2", 0, (0, 1, 2, 3)),
        ("l3p", 0), ("l3p", 1), ("l3p", 2), ("l3p", 3),
    ],
    "q_eng": 'V',        # final psq->sbuf copy engine
    "t0_dma": ("cb0a", "cb0b", "bc", "w8P", "w2s8"),
    "pe_warm": True,
    "early_drain": True,
    "t_dma": ("cb", "w8"),
}
# The last tile has no successor phase: drain it inside its own phase.
CFG["template_last"] = CFG["template"] + [
    ("qdA",),
    ("l2", 1, (0, 1, 2, 3)),
    ("l2", 2, (0, 1, 2, 3)),
    ("rhff", 0), ("rhff", 1), ("rhff", 2), ("rhff", 3),
    ("l3", 0), ("l3", 1), ("l3", 2), ("l3", 3),
]

_NC = None


def build_nc():
    nc = bacc.Bacc("TRN2", target_bir_lowering=False, debug=False,
                   num_devices=NCORES)

    comb16 = nc.declare_dram_parameter("comb16", [NT, 128, CB], BF16,
                                       isOutput=False)
    # per-tile fp8 blob [128, 3, 512]: sub0 = action rows (rhs k-tile),
    # sub1 = action lhsT block, sub2 = W2s k-tile-4 lhsT
    comb8 = nc.declare_dram_parameter("comb8", [NT, 128, NP2, T], FP8,
                                      isOutput=False)
    # shared L2 pairs 0,1 lhsT: [m, p(2), i(2), col(128)]
    w2s8 = nc.declare_dram_parameter("w2s8", [128, M2, 2, 2, 128], FP8,
                                     isOutput=False)
    # b1 biases (*SH): cols t*K1+c
    bconst = nc.declare_dram_parameter("bconst", [128, NT * K1], F32,
                                       isOutput=False)
    # q[j, 4t+s] = 2048*q of lane 512t + 128s + j
    q = nc.declare_dram_parameter("q", [128, M2 * NT], F32, isOutput=True)

    eng = {'A': lambda: nc.scalar, 'V': lambda: nc.vector,
           'P': lambda: nc.gpsimd}

    with tile.TileContext(nc) as tc:
        with (
            tc.tile_pool(name="const", bufs=1) as const,
            tc.tile_pool(name="wts", bufs=4) as wts,
            tc.tile_pool(name="hpool", bufs=3) as hpool,
            tc.tile_pool(name="outp", bufs=1) as outp,
            tc.tile_pool(name="ps1", bufs=CFG["ps1_bufs"], space="PSUM") as ps1p,
            tc.tile_pool(name="ps2", bufs=CFG["ps2_bufs"], space="PSUM") as ps2p,
            tc.tile_pool(name="psq", bufs=1, space="PSUM") as psqp,
        ):
            w2s8_sb = const.tile([128, M2, 2, 2, 128], FP8)
            bc_sb = const.tile([128, NT * K1], F32)
            psq = psqp.tile([128, M2 * NT], F32)
            # Warm-up: trigger the ACT-table load (~1.3us) during the
            # initial DMA dead time instead of before the first real relu.
            dumt = const.tile([1, 4], F32)
            nc.gpsimd.memset(dumt[:], 0.0)
            nc.scalar.activation(dumt[:, 0:1], dumt[:, 1:2], RELU, bias=0.0)
            if CFG.get("pe_warm"):
                nc.tensor.matmul(psq[0:1, 0:1], dumt[0:1, 2:3],
                                 dumt[0:1, 3:4], start=True, stop=True)

            def relu_op(e, out, in_, bias):
                if e == 'A':
                    nc.scalar.activation(out, in_, RELU, bias=bias)
                elif isinstance(bias, float):
                    eng[e]().tensor_scalar_max(out, in_, bias)
                else:
                    eng[e]().tensor_scalar(out, in_, bias, 0.0,
                                           mybir.AluOpType.add,
                                           mybir.AluOpType.max)

            class Tile:
                def __init__(self, t):
                    self.t = t
                    self.cb = wts.tile([128, CB], BF16, tag="cb")
                    # w8: subs 0:3 DMA'd (action rows, action lhsT, W2s kt4);
                    # sub 3 = 32*h1 k-tile 4, written by the Pool relu.
                    self.w8 = wts.tile([128, NP2 + 1, T], FP8, tag="w8")
                    self.h1 = hpool.tile([128, 4, T], FP8, tag="h1")
                    dmas = {
                        "cb": lambda: nc.sync.dma_start(self.cb[:], comb16[t]),
                        "cb0a": lambda: nc.sync.dma_start(
                            self.cb[:, 0:W1OFF + 128],
                            comb16[t][:, 0:W1OFF + 128]),
                        "cb0b": lambda: nc.sync.dma_start(
                            self.cb[:, W1OFF + 128:],
                            comb16[t][:, W1OFF + 128:]),
                        "w8": lambda: nc.sync.dma_start(
                            self.w8[:, 0:NP2, :], comb8[t]),
                        "w8P": lambda: nc.gpsimd.dma_start(
                            self.w8[:, 0:NP2, :], comb8[t]),
                        "w2s8": lambda: nc.sync.dma_start(
                            w2s8_sb[:], w2s8.ap()),
                        "bc": lambda: nc.sync.dma_start(bc_sb[:], bconst.ap()),
                    }
                    order = CFG["t0_dma"] if t == 0 else CFG["t_dma"]
                    for d in order:
                        dmas[d]()
                    self.hf = hpool.tile([128, M2, T], BF16, tag="hf")
                    self.ps1 = {}
                    self.ps2 = {}

                def l1(self, c):
                    ps1c = ps1p.tile([128, T], F32, tag="ps1")
                    p = self.ps1[c] = ps1c
                    nc.tensor.matmul(p[:], self.cb[:, W1OFF + 128 * c:
                                                   W1OFF + 128 * (c + 1)],
                                     self.cb[:, STOFF:STOFF + T],
                                     start=True, stop=True)

                def r1(self, c, e):
                    p = self.ps1.pop(c)
                    b = bc_sb[:, self.t * K1 + c:self.t * K1 + c + 1]
                    out = (self.w8[:, NP2, :] if c == K1 - 1
                           else self.h1[:, c, :])
                    relu_op(e, out, p[:], b)

                def l2(self, p, ms):
                    for m in ms:
                        if p == 0:
                            ps2m = ps2p.tile([128, T], F32, tag="ps2")
                            self.ps2[m] = ps2m
                        if p == NP2 - 1:
                            lhsT = self.w8[:, 1:NP2, 128 * m:128 * (m + 1)]
                            rhs = self.w8[:, 0:NP2 + 1:NP2, :]
                        else:
                            lhsT = w2s8_sb[:, m, p, :, :]
                            rhs = self.h1[:, 2 * p:2 * p + 2, :]
                        nc.tensor.matmul(self.ps2[m][:], lhsT, rhs,
                                         start=(p == 0), stop=(p == NP2 - 1),
                                         perf_mode=DR)

                def rhf(self, m, e):
                    p = self.ps2.pop(m)
                    hfm = self.hf[:, m, :]
                    if isinstance(e, tuple):
                        e0, h, e1 = e
                        relu_op(e0, hfm[:, 0:h], p[:, 0:h], 0.0)
                        relu_op(e1, hfm[:, h:T], p[:, h:T], 0.0)
                    else:
                        relu_op(e, hfm, p[:], 0.0)

                def l3(self, s):
                    col = M2 * self.t + s
                    for m in range(M2):
                        nc.tensor.matmul(psq[:, col:col + 1],
                                         self.hf[:, m, ts(s, 128)],
                                         self.cb[:, W3OFF + m:W3OFF + m + 1],
                                         start=(m == 0), stop=(m == M2 - 1))

            q_sb = outp.tile([128, M2 * NT], F32, tag="q")
            ned = M2 * (NT - 1)

            def drain_a():
                eng[CFG["q_eng"]]().tensor_copy(q_sb[:, 0:ned],
                                                psq[:, 0:ned])
                nc.sync.dma_start(q.ap()[:, 0:ned], q_sb[:, 0:ned])

            def run_token(tok, cur, prev, flush=False):
                kind = tok[0]
                if kind == "l1":
                    cur.l1(tok[1])
                elif kind == "r1":
                    cur.r1(tok[1], CFG["r1_eng"].get(tok[1]))
                elif kind == "l2":
                    cur.l2(tok[1], tok[2])
                elif kind == "l2p":
                    if prev is not None:
                        prev.l2(tok[1], tok[2])
                elif kind == "rhf":
                    cur.rhf(tok[1], CFG["rhf_eng"][tok[1]])
                elif kind == "rhff":
                    cur.rhf(tok[1], CFG["flush_rhf_eng"][tok[1]])
                elif kind == "l3":
                    cur.l3(tok[1])
                elif kind == "rhfp":
                    if prev is not None:
                        emap = CFG["flush_rhf_eng"] if flush else CFG["rhf_eng"]
                        prev.rhf(tok[1], emap[tok[1]])
                elif kind == "l3p":
                    if prev is not None:
                        prev.l3(tok[1])
                elif kind == "qdA":
                    drain_a()
                else:
                    raise ValueError(tok)

            prev = None
            nxt = Tile(0)
            for t in range(NT):
                cur = nxt
                # prefetch: issue tile t+1's DMAs before phase t's tokens
                nxt = Tile(t + 1) if t + 1 < NT else None
                tmpl = CFG["template"]
                if t == NT - 1 and CFG.get("template_last"):
                    tmpl = CFG["template_last"]
                for tok in tmpl:
                    run_token(tok, cur, prev)
                prev = cur
            e = CFG["qb_eng"]
            if e == 'A':
                nc.scalar.activation(q_sb[:, ned:], psq[:, ned:],
                                     mybir.ActivationFunctionType.Copy)
            else:
                eng[e]().tensor_copy(q_sb[:, ned:], psq[:, ned:])
            nc.sync.dma_start(q.ap()[:, ned:], q_sb[:, ned:])

    nc.compile()
    return nc


def _get_nc():
    global _NC
    if _NC is None:
        _NC = build_nc()
    return _NC


def _plan_tiles(idx):
    """Stable-sort samples by game, pad each game segment to 512-sample
    tiles, pad the tile list to the fixed 72. Returns (sel, valid, gids):
    sel[t, l] = original sample index feeding lane l of tile t."""
    perm = np.argsort(idx, kind="stable")
    counts = np.bincount(idx, minlength=G)
    ntot = NCORES * NT
    sel = np.zeros((ntot, T), np.int64)
    valid = np.zeros((ntot, T), bool)
    gids = np.zeros(ntot, np.int64)
    pos, t = 0, 0
    for g in range(G):
        cg = int(counts[g])
        for k in range((cg + T - 1) // T):
            n = min(T, cg - k * T)
            lanes = perm[pos:pos + n]
            sel[t, :n] = lanes
            valid[t, :n] = True
            if n < T:
                sel[t, n:] = lanes[0]
            gids[t] = g
            pos += n
            t += 1
    assert t <= ntot, f"tile plan overflow: {t} > {ntot}"
    return sel, valid, gids


def _f8(x):
    return np.asarray(x, np.float32).astype(_NP_FP8)


def build_in_maps(inputs):
    state = np.ascontiguousarray(np.asarray(inputs["state"], np.float32))
    action = np.ascontiguousarray(np.asarray(inputs["action"], np.float32))
    idx = np.asarray(inputs["idx"]).astype(np.int64)
    W1 = np.asarray(inputs["W1"], np.float32)
    b1 = np.asarray(inputs["b1"], np.float32)
    W2s = np.asarray(inputs["W2s"], np.float32)
    b2s = np.asarray(inputs["b2s"], np.float32)
    W2a = np.asarray(inputs["W2a"], np.float32)
    W3 = np.asarray(inputs["W3"], np.float32)
    assert state.shape == (B, D) and action.shape == (B, A)

    sel, valid, gids = _plan_tiles(idx)

    # L1 weights: 32*W1^T in bf16, padded 600->640.
    W1T_all = np.zeros((G, D, H1P), np.float32)
    W1T_all[:, :, :H1] = W1.transpose(0, 2, 1) * SH
    b1P = np.zeros((G, H1P), np.float32)
    b1P[:, :H1] = b1 * SH
    b1c_all = np.ascontiguousarray(b1P.reshape(G, K1, 128).transpose(0, 2, 1))

    # L2 shared weights: fp8(64*W2s^T), k-tiles arranged per (m, pair, i).
    W2sTP = np.zeros((H1P, H2P), np.float32)
    W2sTP[:H1, :H2] = W2s.T * SW
    W2sq = _f8(W2sTP)  # [640, 512]
    w2skt = W2sq.reshape(K1, 128, M2, 128)
    w2s8_host = np.zeros((128, M2, 2, 2, 128), _NP_FP8)
    for p in range(2):
        for i in range(2):
            w2s8_host[:, :, p, i, :] = w2skt[2 * p + i]

    # Per-game action lhsT block rows [Whi(16); Whi(16); Wlo(16); 64*b2s].
    W2aT_all = np.zeros((G, A, H2P), np.float32)
    W2aT_all[:, :, :H2] = W2a.transpose(0, 2, 1) * SW
    Wahi = _f8(W2aT_all)
    Walo = _f8(W2aT_all - Wahi.astype(np.float32))
    b2sP = np.zeros(H2P, np.float32)
    b2sP[:H2] = b2s * SW
    b2sq = _f8(b2sP)
    act_lhsT = np.zeros((G, 128, T), _NP_FP8)
    act_lhsT[:, 0:A] = Wahi
    act_lhsT[:, A:2 * A] = Wahi
    act_lhsT[:, 2 * A:3 * A] = Walo
    act_lhsT[:, 3 * A] = b2sq[None, :]

    W3P = np.zeros((G, H2P), np.float32)
    W3P[:, :H2] = W3
    W3T_all = np.ascontiguousarray(W3P.reshape(G, M2, 128).transpose(0, 2, 1))

    # cb per-game static part: [128, w1t | w3t] bf16 (state filled per tile)
    wb16_all = np.zeros((G, 128, H1P + M2), np.float32)
    wb16_all[:, :, 0:H1P] = W1T_all
    wb16_all[:, :, H1P:] = W3T_all
    wb16_all = wb16_all.astype(_NP_BF16)

    # Action rows (per sample): [ahi; alo; ahi; 32.0; 0...] of 32*action^T.
    aS = action * SH
    ahi_all = _f8(aS)                                    # [B, 16]
    alo_all = _f8(aS - ahi_all.astype(np.float32))       # [B, 16]

    in_maps = []
    for c in range(NCORES):
        tsl = slice(c * NT, (c + 1) * NT)
        lanes = sel[tsl].reshape(-1)
        gt = gids[tsl]
        bconst = np.ascontiguousarray(
            b1c_all[gt].transpose(1, 0, 2).reshape(128, NT * K1))
        comb16_host = np.zeros((NT, 128, CB), _NP_BF16)
        comb16_host[:, :, STOFF:STOFF + T] = (
            state[lanes].T.reshape(128, NT, T).transpose(1, 0, 2)
            .astype(_NP_BF16))
        comb16_host[:, :, W1OFF:] = wb16_all[gt]
        comb8_host = np.zeros((NT, 128, NP2, T), _NP_FP8)
        ahi_c = ahi_all[lanes].reshape(NT, T, A)
        alo_c = alo_all[lanes].reshape(NT, T, A)
        for t in range(NT):
            comb8_host[t, 0:A, 0] = ahi_c[t].T
            comb8_host[t, A:2 * A, 0] = alo_c[t].T
            comb8_host[t, 2 * A:3 * A, 0] = ahi_c[t].T
            comb8_host[t, 3 * A, 0] = np.float32(SH)
        comb8_host[:, :, 1] = act_lhsT[gt]
        comb8_host[:, :, 2] = w2skt[4].reshape(128, T)[None]
        in_maps.append({
            "comb16": comb16_host,
            "comb8": comb8_host,
            "w2s8": w2s8_host,
            "bconst": bconst,
        })
    return in_maps, sel, valid


def kernel(**inputs):
    idx = np.asarray(inputs["idx"]).astype(np.int64)
    b3 = np.asarray(inputs["b3"], np.float32)
    in_maps, sel, valid = build_in_maps(inputs)

    res = run_bass_kernel_spmd(_get_nc(), in_maps, list(range(NCORES))).results
    # q[j, 4t+s] = lane 512t + 128s + j  ->  [t, s, j] order
    qv = np.concatenate([
        np.asarray(res[c]["q"]).reshape(128, NT, M2).transpose(1, 2, 0)
        .reshape(-1)
        for c in range(NCORES)])

    out = np.zeros(B, np.float32)
    flat_sel = sel.reshape(-1)
    flat_valid = valid.reshape(-1)
    out[flat_sel[flat_valid]] = qv[flat_valid] / SP2
    out += b3[idx]
    return out.astype(np.float32)
